# revision 13
# baseline (speedup 1.0000x reference)
"""Trainium2 Bass kernel for 3x3 VALID conv: x[32,128,64,64] * w[256,128,3,3] + bias.

Strategy:
  - Data-parallel over batch: 8 cores x 4 images each; weights/bias replicated.
  - Per core: implicit GEMM. Contraction dim = C_IN = 128 = partition dim.
    For each filter tap (u,v), accumulate
        psum[o, i, j] += W[c, o; u,v].T @ x[c, i+u, j+v]
    with the moving operand a strided [r, 62] view of a [C, rows, W] input
    piece, so only the 62 valid output columns are streamed.
  - bf16 x/w (accuracy ~2e-3 << 2e-2 tol): halves DMA bytes and SBUF
    bandwidth; matmul still 1 cycle/row. PSUM accumulates fp32.
  - Tile-granular dependencies drive the layout: weights split into 3 tiles
    (taps 0-2 half0 / taps 3-8 half0 / half1) and x into per-2-chunk row
    pieces, so the first matmuls wait only on the minimal critical DMAs.
  - Dummy matmuls on a zeroed scratch tile ramp the PE DVFS p-state while
    the first DMAs land; any stream gap would otherwise reset the ramp.
  - Rings: x loads on the Sync HWDGE ring; weights/bias/y stores on the
    ScalarE HWDGE ring.
"""

import numpy as np
import ml_dtypes

import concourse.bacc as bacc
import concourse.tile as tile
from concourse import mybir
from concourse.bass_utils import run_bass_kernel_spmd

N_CORES = 8
B_FULL, C_IN, H, W = 32, 128, 64, 64
C_OUT, KH, KW = 256, 3, 3
B_LOC = B_FULL // N_CORES          # images per core
H_OUT = W_OUT = H - KH + 1         # 62
N_HALF = C_OUT // 128              # 2 output-channel halves
RPC = 8                            # output rows per PSUM chunk (8*62 <= 512)
N_CHUNKS = (H_OUT + RPC - 1) // RPC
P_ROWS = 2 * RPC + KH - 1          # input rows per 2-chunk x piece (18)
N_PIECES = 4                       # pieces [0:18],[16:34],[32:50],[48:64]
N_WARM = 6                         # DVFS warm-up matmuls

_cached = {}


def _build_nc():
    f32 = mybir.dt.float32
    bf16 = mybir.dt.bfloat16
    nc = bacc.Bacc()

    x_d = nc.declare_dram_parameter("x", [B_LOC, C_IN, H, W], bf16, isOutput=False)
    w_d = nc.declare_dram_parameter(
        "w", [C_IN, N_HALF, KH * KW, 128], bf16, isOutput=False
    )
    b_d = nc.declare_dram_parameter("bias_in", [128, N_HALF], f32, isOutput=False)
    y_d = nc.declare_dram_parameter(
        "y", [B_LOC, N_HALF, 128, H_OUT, W_OUT], bf16, isOutput=True
    )

    with tile.TileContext(nc) as tc:
        with (
            tc.tile_pool(name="const", bufs=1) as cpool,
            tc.tile_pool(name="xin", bufs=5) as xpool,
            tc.tile_pool(name="out", bufs=4) as opool,
            tc.tile_pool(name="psum", bufs=4, space="PSUM") as ppool,
            tc.tile_pool(name="warm", bufs=1, space="PSUM") as wpool,
        ):
            # Weights split by consumption order (tile-granular deps).
            w_a = cpool.tile([C_IN, 3, 128], bf16)        # half0 taps 0-2
            w_b = cpool.tile([C_IN, KH * KW - 3, 128], bf16)  # half0 taps 3-8
            w_c = cpool.tile([C_IN, KH * KW, 128], bf16)  # half1 all taps
            b_t = cpool.tile([128, N_HALF], f32)
            scr = cpool.tile([128, 512], bf16)

            nc.gpsimd.memset(scr[:], 0.0)

            # Critical-path DMAs for the first matmul group.
            x_p0 = xpool.tile([C_IN, P_ROWS, W], bf16, tag="x")
            nc.sync.dma_start(x_p0[:], x_d[0, :, 0:P_ROWS, :])
            nc.sync.dma_start(w_a[:], w_d[:, 0, 0:3])
            nc.sync.dma_start(w_b[:], w_d[:, 0, 3 : KH * KW])
            nc.sync.dma_start(w_c[:], w_d[:, 1])
            nc.scalar.dma_start(b_t[:], b_d[:])

            # Ramp the PE p-state while those land (distinct sizes so no
            # two warm-up matmuls are identical instructions).
            pwarm = wpool.tile([128, 512], f32)
            for i in range(N_WARM):
                nc.tensor.matmul(
                    pwarm[:, 0 : 512 - i],
                    scr[:, 0:128],
                    scr[:, 0 : 512 - i],
                    start=True,
                    stop=True,
                )

            def lhsT(half, uv):
                if half == 1:
                    return w_c[:, uv, :]
                return w_a[:, uv, :] if uv < 3 else w_b[:, uv - 3, :]

            def load_piece(b, k, eng):
                px = xpool.tile([C_IN, P_ROWS, W], bf16, tag="x")
                r0 = 2 * RPC * k
                nr = min(P_ROWS, H - r0)
                eng.dma_start(px[:, 0:nr, :], x_d[b, :, r0 : r0 + nr, :])
                return px

            for b in range(B_LOC):
                # Prefetch this image's pieces. Image 0's trailing pieces ride
                # the Scalar ring BEHIND the weight DMAs so the critical w_b/
                # w_c descriptors aren't queued behind bulk prefetch; later
                # images prefetch on the otherwise-idle Sync ring.
                eng = nc.scalar if b == 0 else nc.sync
                piece_tiles = [
                    x_p0 if (b == 0 and k == 0) else load_piece(b, k, eng)
                    for k in range(N_PIECES)
                ]
                for chunk in range(N_CHUNKS):
                    i0 = chunk * RPC
                    r = min(RPC, H_OUT - i0)
                    k = chunk // 2
                    px = piece_tiles[k]
                    li = i0 - 2 * RPC * k   # row offset within the piece
                    for half in range(N_HALF):
                        ps = ppool.tile([128, RPC, W_OUT], f32, tag="ps")
                        for uv in range(KH * KW):
                            u, v = divmod(uv, KW)
                            nc.tensor.matmul(
                                ps[:, 0:r, :],
                                lhsT(half, uv),
                                px[:, li + u : li + u + r, v : v + W_OUT],
                                start=(uv == 0),
                                stop=(uv == KH * KW - 1),
                            )
                        o_t = opool.tile([128, RPC, W_OUT], bf16, tag="o")
                        nc.vector.tensor_scalar_add(
                            o_t[:, 0:r, :], ps[:, 0:r, :], b_t[:, half : half + 1]
                        )
                        nc.scalar.dma_start(
                            y_d[b, half, :, i0 : i0 + r, :], o_t[:, 0:r, :]
                        )

    nc.compile()
    if not nc.is_finalized():
        nc.finalize()
    return nc


def kernel(inputs, weights, bias, profile=False, trace_kwargs=None):
    x_b = np.ascontiguousarray(
        np.asarray(inputs, dtype=np.float32).astype(ml_dtypes.bfloat16)
    )
    # [O, C, KH, KW] -> [C, half, KH*KW, o_local]  (lhsT layout: contraction dim
    # on partitions; each half contiguous per partition for fast DMA)
    w_t = np.ascontiguousarray(
        np.asarray(weights, dtype=np.float32)
        .reshape(N_HALF, 128, C_IN, KH * KW)
        .transpose(2, 0, 3, 1)
        .astype(ml_dtypes.bfloat16)
    )
    # [C_OUT, 1] -> [128, N_HALF] with bias_sb[p, h] = bias[h*128 + p]
    b_t = np.ascontiguousarray(
        np.asarray(bias, dtype=np.float32).reshape(N_HALF, 128).T
    )

    if "nc" not in _cached:
        _cached["nc"] = _build_nc()
    nc = _cached["nc"]

    in_maps = [
        {
            "x": x_b[i * B_LOC : (i + 1) * B_LOC],
            "w": w_t,
            "bias_in": b_t,
        }
        for i in range(N_CORES)
    ]
    res = run_bass_kernel_spmd(
        nc,
        in_maps,
        list(range(N_CORES)),
        trace=profile,
        **(trace_kwargs or {}),
    )
    _cached["last_result"] = res

    shards = []
    for i in range(N_CORES):
        y = res.results[i]["y"]  # [B_LOC, 2, 128, 62, 62] bf16
        shards.append(
            np.asarray(y).astype(np.float32).reshape(B_LOC, C_OUT, H_OUT, W_OUT)
        )
    return np.ascontiguousarray(np.concatenate(shards, axis=0), dtype=np.float32)


# revision 17
# speedup vs baseline: 1.0045x; 1.0045x over previous
"""Trainium2 Bass kernel for 3x3 VALID conv: x[32,128,64,64] * w[256,128,3,3] + bias.

Strategy:
  - Data-parallel over batch: 8 cores x 4 images each; weights/bias replicated.
  - Per core: implicit GEMM. Contraction dim = C_IN = 128 = partition dim.
    For each filter tap (u,v), accumulate
        psum[o, i, j] += W[c, o; u,v].T @ x[c, i+u, j+v]
    with the moving operand a strided [r, 62] view of a [C, rows, W] input
    piece, so only the 62 valid output columns are streamed.
  - bf16 x/w (accuracy ~2e-3 << 2e-2 tol): halves DMA bytes and SBUF
    bandwidth; matmul still 1 cycle/row. PSUM accumulates fp32.
  - Tile-granular dependencies drive the layout: weights split into 3 tiles
    (taps 0-2 half0 / taps 3-8 half0 / half1) and x into per-2-chunk row
    pieces, so the first matmuls wait only on the minimal critical DMAs.
  - Dummy matmuls on a zeroed scratch tile ramp the PE DVFS p-state while
    the first DMAs land; any stream gap would otherwise reset the ramp.
  - Rings: x loads on the Sync HWDGE ring; weights/bias/y stores on the
    ScalarE HWDGE ring.
"""

import numpy as np
import ml_dtypes

import concourse.bacc as bacc
import concourse.tile as tile
from concourse import mybir
from concourse.bass_utils import run_bass_kernel_spmd

N_CORES = 8
B_FULL, C_IN, H, W = 32, 128, 64, 64
C_OUT, KH, KW = 256, 3, 3
B_LOC = B_FULL // N_CORES          # images per core
H_OUT = W_OUT = H - KH + 1         # 62
N_HALF = C_OUT // 128              # 2 output-channel halves
RPC = 8                            # output rows per PSUM chunk (8*62 <= 512)
N_CHUNKS = (H_OUT + RPC - 1) // RPC
P_ROWS = 2 * RPC + KH - 1          # input rows per 2-chunk x piece (18)
N_PIECES = 4                       # pieces [0:18],[16:34],[32:50],[48:64]
N_WARM = 8                         # DVFS warm-up matmuls

_cached = {}


def _build_nc():
    f32 = mybir.dt.float32
    bf16 = mybir.dt.bfloat16
    nc = bacc.Bacc()

    x_d = nc.declare_dram_parameter("x", [B_LOC, C_IN, H, W], bf16, isOutput=False)
    w_d = nc.declare_dram_parameter(
        "w", [C_IN, N_HALF, KH * KW, 128], bf16, isOutput=False
    )
    b_d = nc.declare_dram_parameter("bias_in", [128, N_HALF], f32, isOutput=False)
    y_d = nc.declare_dram_parameter(
        "y", [B_LOC, N_HALF, 128, H_OUT, W_OUT], bf16, isOutput=True
    )

    with tile.TileContext(nc) as tc:
        with (
            tc.tile_pool(name="const", bufs=1) as cpool,
            tc.tile_pool(name="xin", bufs=5) as xpool,
            tc.tile_pool(name="out", bufs=4) as opool,
            tc.tile_pool(name="psum", bufs=4, space="PSUM") as ppool,
            tc.tile_pool(name="warm", bufs=1, space="PSUM") as wpool,
        ):
            # One weight tile per half (tile-granular deps: the first matmul
            # of a half waits for exactly that half's 9 taps).
            w_h0 = cpool.tile([C_IN, KH * KW, 128], bf16)
            w_h1 = cpool.tile([C_IN, KH * KW, 128], bf16)
            b_t = cpool.tile([128, N_HALF], f32)
            scr = cpool.tile([128, 512], bf16)

            nc.gpsimd.memset(scr[:], 0.0)

            # Critical-path DMAs for the first matmul group.
            x_p0 = xpool.tile([C_IN, P_ROWS, W], bf16, tag="x")
            nc.sync.dma_start(x_p0[:], x_d[0, :, 0:P_ROWS, :])
            nc.scalar.dma_start(w_h0[:], w_d[:, 0])
            nc.scalar.dma_start(w_h1[:], w_d[:, 1])
            nc.scalar.dma_start(b_t[:], b_d[:])

            # Ramp the PE p-state while those land (distinct sizes so no
            # two warm-up matmuls are identical instructions).
            pwarm = wpool.tile([128, 512], f32)
            for i in range(N_WARM):
                nc.tensor.matmul(
                    pwarm[:, 0 : 512 - i],
                    scr[:, 0:128],
                    scr[:, 0 : 512 - i],
                    start=True,
                    stop=True,
                )

            def lhsT(half, uv):
                return (w_h1 if half else w_h0)[:, uv, :]

            def load_piece(b, k, eng):
                px = xpool.tile([C_IN, P_ROWS, W], bf16, tag="x")
                r0 = 2 * RPC * k
                nr = min(P_ROWS, H - r0)
                eng.dma_start(px[:, 0:nr, :], x_d[b, :, r0 : r0 + nr, :])
                return px

            for b in range(B_LOC):
                # Prefetch this image's pieces on the Sync ring; the Scalar
                # ring carries only weights/bias (critical) and y stores.
                eng = nc.sync
                piece_tiles = [
                    x_p0 if (b == 0 and k == 0) else load_piece(b, k, eng)
                    for k in range(N_PIECES)
                ]
                for chunk in range(N_CHUNKS):
                    i0 = chunk * RPC
                    r = min(RPC, H_OUT - i0)
                    k = chunk // 2
                    px = piece_tiles[k]
                    li = i0 - 2 * RPC * k   # row offset within the piece
                    for half in range(N_HALF):
                        ps = ppool.tile([128, RPC, W_OUT], f32, tag="ps")
                        for uv in range(KH * KW):
                            u, v = divmod(uv, KW)
                            nc.tensor.matmul(
                                ps[:, 0:r, :],
                                lhsT(half, uv),
                                px[:, li + u : li + u + r, v : v + W_OUT],
                                start=(uv == 0),
                                stop=(uv == KH * KW - 1),
                            )
                        o_t = opool.tile([128, RPC, W_OUT], bf16, tag="o")
                        nc.vector.tensor_scalar_add(
                            o_t[:, 0:r, :], ps[:, 0:r, :], b_t[:, half : half + 1]
                        )
                        nc.scalar.dma_start(
                            y_d[b, half, :, i0 : i0 + r, :], o_t[:, 0:r, :]
                        )

    nc.compile()
    if not nc.is_finalized():
        nc.finalize()
    return nc


def kernel(inputs, weights, bias, profile=False, trace_kwargs=None):
    x_b = np.ascontiguousarray(
        np.asarray(inputs, dtype=np.float32).astype(ml_dtypes.bfloat16)
    )
    # [O, C, KH, KW] -> [C, half, KH*KW, o_local]  (lhsT layout: contraction dim
    # on partitions; each half contiguous per partition for fast DMA)
    w_t = np.ascontiguousarray(
        np.asarray(weights, dtype=np.float32)
        .reshape(N_HALF, 128, C_IN, KH * KW)
        .transpose(2, 0, 3, 1)
        .astype(ml_dtypes.bfloat16)
    )
    # [C_OUT, 1] -> [128, N_HALF] with bias_sb[p, h] = bias[h*128 + p]
    b_t = np.ascontiguousarray(
        np.asarray(bias, dtype=np.float32).reshape(N_HALF, 128).T
    )

    if "nc" not in _cached:
        _cached["nc"] = _build_nc()
    nc = _cached["nc"]

    in_maps = [
        {
            "x": x_b[i * B_LOC : (i + 1) * B_LOC],
            "w": w_t,
            "bias_in": b_t,
        }
        for i in range(N_CORES)
    ]
    res = run_bass_kernel_spmd(
        nc,
        in_maps,
        list(range(N_CORES)),
        trace=profile,
        **(trace_kwargs or {}),
    )
    _cached["last_result"] = res

    shards = []
    for i in range(N_CORES):
        y = res.results[i]["y"]  # [B_LOC, 2, 128, 62, 62] bf16
        shards.append(
            np.asarray(y).astype(np.float32).reshape(B_LOC, C_OUT, H_OUT, W_OUT)
        )
    return np.ascontiguousarray(np.concatenate(shards, axis=0), dtype=np.float32)
